# revision 18
# baseline (speedup 1.0000x reference)
"""Trainium2 Bass kernel for DigitConvolutionalModel (conv3x3 -> fc 676x128 -> relu -> fc 128x10).

Strategy
--------
The 3x3 valid conv with a replicated 3x3 weight is a linear map, so
    conv(x).reshape(B, 676) @ w1  ==  x @ W1eff,
where W1eff[784, 128] is assembled on the host from conv_w and w1 (68 MFLOP,
negligible). The device work is then a fused 2-layer MLP:
    out = relu(x @ W1eff + b1) @ w2 + b2.

Sharding: pure data parallel over 8 NeuronCores, 2048 batch rows per core.

Device-side layout choices (all driven by profile evidence):
 - The host pre-arranges x into the exact SBUF image each DMA writes:
   per core, xdev[nb][p][c*512+j] = x[nb*512+j, c*128+p] for the six full
   128-pixel contraction chunks, plus a separate [16, 2048] tail array for
   pixels 768..784. Every DMA is then partition-contiguous (7 KB runs), which
   cuts HWDGE descriptor-generation (issue) time and runs near line rate.
 - Weights/biases are packed into 2 DMAs (one fp16 blob, one fp32 blob) and
   issued on the Scalar engine's HWDGE queue so they don't serialize with the
   x-block DMAs on the Sync queue.
 - fc1 accumulates 7 matmuls into a PSUM bank (bufs=3 so the PE never waits
   on the activation drain); relu+b1 on ScalarE; fc2 on PE; +b2 on VectorE
   (keeps ScalarE's strict FIFO free for the next block's relu); one output
   DMA at the end.
"""

import os
import sys

import numpy as np

_TRN_REPO = "/opt/trn_rl_repo"
if _TRN_REPO not in sys.path:
    sys.path.insert(0, _TRN_REPO)

import concourse.bass as bass  # noqa: E402
import concourse.bacc as bacc  # noqa: E402
import concourse.mybir as mybir  # noqa: E402
import concourse.tile as tile  # noqa: E402
from concourse.bass_utils import run_bass_kernel_spmd  # noqa: E402

N_CORES = 8
B = 16384
BC = B // N_CORES  # 2048 batch rows per core
NPIX = 784  # 28*28 input pixels
C6 = 6  # full 128-row contraction chunks
KT = NPIX - C6 * 128  # 16-row tail chunk
NF1 = 128
NF2 = 10
NBLK = 512  # batch block = one PSUM bank of fp32
NB = BC // NBLK

# wpack free-dim layout: [c*128 : (c+1)*128] = w1 chunk c (c<6),
# [768:896] = w1 tail (first 16 partitions), [896:906] = w2.
WPACK_W = C6 * 128 + 128 + NF2

# x DMA pieces (start, width) == compute blocks; the final piece is tiny so
# almost no compute remains once the last x semaphore fires.
XPIECES = [(0, 512), (512, 512), (1024, 512), (1536, 384), (1920, 128)]
CBLOCKS = XPIECES

_DT_NAME = os.environ.get("DIGIT_DT", "float16")
DT = getattr(mybir.dt, _DT_NAME)
DT_NP = mybir.dt.np(DT)

_NC_CACHE = None


def _build_nc():
    nc = bacc.Bacc(
        "TRN2", target_bir_lowering=False, debug=False, num_devices=N_CORES
    )
    xdev = nc.dram_tensor("xdev", [128, C6 * BC], DT, kind="ExternalInput").ap()
    xtail = nc.dram_tensor("xtail", [KT, BC], DT, kind="ExternalInput").ap()
    wpack = nc.dram_tensor("wpack", [128, WPACK_W], DT, kind="ExternalInput").ap()
    bpack = nc.dram_tensor(
        "bpack", [128, 2], mybir.dt.float32, kind="ExternalInput"
    ).ap()
    outT = nc.dram_tensor(
        "outT", [NF2, BC], mybir.dt.float32, kind="ExternalOutput"
    ).ap()

    with tile.TileContext(nc) as tc:
        with (
            tc.tile_pool(name="w", bufs=1) as wpool,
            tc.tile_pool(name="xin", bufs=1) as xpool,
            tc.tile_pool(name="h", bufs=3) as hpool,
            tc.tile_pool(name="o", bufs=1) as opool,
            tc.tile_pool(name="ps1", bufs=4, space=bass.MemorySpace.PSUM) as ps1pool,
            tc.tile_pool(name="ps2", bufs=2, space=bass.MemorySpace.PSUM) as ps2pool,
        ):
            # x blocks back-to-back on the Sync HWDGE queue; everything the
            # early matmuls also need (weights, tail, biases) rides the
            # Scalar HWDGE queue in parallel.
            # x pieces on Sync. Total DMA count stays at 8 unique sem lanes
            # (4 x + 3 scalar-queue + final out; the early out recycles a
            # long-consumed lane) — more DMAs than lanes blocks the issue
            # queue on lane recycling.
            xsb = []
            for bn, (s0, w) in enumerate(XPIECES):
                t = xpool.tile([128, C6, w], DT, tag=f"x{bn}")
                nc.sync.dma_start(
                    t[:],
                    xdev[:, C6 * s0 : C6 * (s0 + w)].rearrange(
                        "p (c n) -> p c n", c=C6
                    ),
                )
                xsb.append(t)

            # weights/tail/biases on the Scalar HWDGE queue (no ACT compute
            # in this kernel, so the queue is dedicated to these DMAs)
            wsb = wpool.tile([128, WPACK_W], DT)
            nc.scalar.dma_start(wsb[:], wpack[:])
            xtsb = xpool.tile([KT, BC], DT, tag="xt")
            nc.scalar.dma_start(xtsb[:], xtail[:])
            bsb = wpool.tile([128, 2], mybir.dt.float32)
            nc.scalar.dma_start(bsb[:], bpack[:])

            osb = opool.tile([NF2, BC], mybir.dt.float32)

            for bn, (s0, w) in enumerate(CBLOCKS):
                xp, j0 = bn, 0
                ps1 = ps1pool.tile([NF1, w], mybir.dt.float32, tag="ps1")
                for c in range(C6):
                    nc.tensor.matmul(
                        ps1[:],
                        wsb[:, bass.ts(c, 128)],
                        xsb[xp][:, c, j0 : j0 + w],
                        start=(c == 0),
                        stop=False,
                    )
                nc.tensor.matmul(
                    ps1[:],
                    wsb[0:KT, C6 * 128 : C6 * 128 + NF1],
                    xtsb[:, s0 : s0 + w],
                    start=False,
                    stop=True,
                )

                # relu + b1 on VectorE: out = max(ps1 + b1, 0)
                hT = hpool.tile([NF1, w], DT, tag="hT")
                nc.vector.tensor_scalar(
                    hT[:],
                    ps1[:],
                    bsb[:, 0:1],
                    0.0,
                    mybir.AluOpType.add,
                    mybir.AluOpType.max,
                )

                ps2 = ps2pool.tile([NF2, w], mybir.dt.float32, tag="ps2")
                nc.tensor.matmul(
                    ps2[:],
                    wsb[:, C6 * 128 + 128 : C6 * 128 + 128 + NF2],
                    hT[:],
                    start=True,
                    stop=True,
                )
                nc.vector.tensor_scalar_add(osb[:, s0 : s0 + w], ps2[:], bsb[0:NF2, 1:2])
                if bn < len(CBLOCKS) - 1:
                    # non-final blocks stream out on the idle Scalar queue,
                    # fully hidden behind the remaining x transfers/compute
                    nc.scalar.dma_start(outT[:, s0 : s0 + w], osb[:, s0 : s0 + w])

            s_last = CBLOCKS[-1][0]
            nc.sync.dma_start(outT[:, s_last:BC], osb[:, s_last:BC])

    nc.compile()
    return nc


def get_nc():
    global _NC_CACHE
    if _NC_CACHE is None:
        _NC_CACHE = _build_nc()
    return _NC_CACHE


def _w1eff(conv_w: np.ndarray, w1: np.ndarray) -> np.ndarray:
    """Fold the 3x3 conv into the fc1 weight: [784, 128] = C @ w1."""
    w1r = np.asarray(w1, np.float32).reshape(26, 26, NF1)
    cw = np.asarray(conv_w, np.float32)
    out = np.zeros((28, 28, NF1), np.float32)
    for di in range(3):
        for dj in range(3):
            out[di : di + 26, dj : dj + 26] += cw[di, dj] * w1r
    return out.reshape(NPIX, NF1)


def make_in_maps(x, conv_w, w1, b1, w2, b2):
    x = np.asarray(x, np.float32)

    w1e = _w1eff(conv_w, w1)
    wpack = np.zeros((128, WPACK_W), np.float32)
    for c in range(C6):
        # SBUF partition p, free slot c*128+f  <-  w1e[c*128+p, f]
        wpack[:, c * 128 : (c + 1) * 128] = w1e[c * 128 : (c + 1) * 128, :]
    wpack[0:KT, C6 * 128 : C6 * 128 + NF1] = w1e[C6 * 128 :, :]
    wpack[:, C6 * 128 + 128 :] = np.asarray(w2, np.float32)
    wpack = wpack.astype(DT_NP)

    bpack = np.zeros((128, 2), np.float32)
    bpack[:, 0] = np.asarray(b1, np.float32)
    bpack[0:NF2, 1] = np.asarray(b2, np.float32)

    # xdev[core][p][C6*s0 + c*w + j] = x[core*2048 + s0 + j, c*128 + p]
    # for each piece (s0, w) — piece layouts are contiguous per DMA.
    xdev = np.empty((N_CORES, 128, C6 * BC), DT_NP)
    xr = x[:, : C6 * 128].reshape(N_CORES, BC, C6, 128)
    for s0, w in XPIECES:
        piece = xr[:, s0 : s0 + w].transpose(0, 3, 2, 1)  # [core, p, c, j]
        xdev[:, :, C6 * s0 : C6 * (s0 + w)] = piece.reshape(N_CORES, 128, C6 * w)
    # xtail[core][p][b] = x[core*2048 + b, 768 + p]
    xt = x[:, C6 * 128 :].reshape(N_CORES, BC, KT)
    xtail = np.ascontiguousarray(xt.transpose(0, 2, 1)).astype(DT_NP)

    in_maps = []
    for i in range(N_CORES):
        in_maps.append(
            {
                "xdev": xdev[i],
                "xtail": xtail[i],
                "wpack": wpack,
                "bpack": bpack,
            }
        )
    return in_maps


def gather_out(results) -> np.ndarray:
    return np.concatenate([np.asarray(r["outT"]).T for r in results], axis=0)


def kernel(x, conv_w, w1, b1, w2, b2) -> np.ndarray:
    nc = get_nc()
    in_maps = make_in_maps(x, conv_w, w1, b1, w2, b2)
    res = run_bass_kernel_spmd(nc, in_maps, list(range(N_CORES)))
    return gather_out(res.results)
